# revision 25
# baseline (speedup 1.0000x reference)
"""GraphSAGE (3x SAGEConv-mean + BN + LeakyReLU) + AvgPool + MLP head on 8 Trainium2
NeuronCores via Bass/Tile.

Sharding: nodes are partitioned contiguously across the 8 cores (2048 each);
weights are replicated; BatchNorm statistics and per-graph pooled sums are
all-reduced; per-layer activations are all-gathered (node-major fp8 in HBM)
so each core can gather the source rows of its incident edges.

Fast path vs the bf16 baseline:
 - Layer-2/3 matmuls (self+neighbor fused into one PSUM accumulation) run in
   fp8e4 with MatmulPerfMode.DoubleRow (2 k-tiles per pass).  BatchNorm makes
   each layer scale-invariant, so weights are pre-scaled by 64 on the host
   for fp8 range with no descale anywhere.  Layer 1 (raw input) stays bf16
   for accuracy; its neighbor mean is fp8.
 - Neighbor aggregation computes m^T = (gathered)^T @ S directly
   (feature-major output, no aggregation transposes); S is an exact one-hot
   fp8 matrix and 1/deg is applied in the PSUM evacuation as an exact bf16
   broadcast multiply.
 - y and m stay SBUF-resident feature-major (no sp/m/rst HBM roundtrips, no
   DMA transposes); node-major fp8 y tiles (stride-2 fp8 PE transposes) are
   written to HBM only for the gather/allgather.
 - Each layer's dense fo-loop is emission-interleaved with the next
   super-chunk's gathers/aggregation so PE, DMA and GPSIMD stream
   continuously; dst-group chunk counts are odd-capable (plain fp8 tail
   matmul) to avoid gather padding.
 - BatchNorm statistics via DVE bn_stats/bn_aggr + a tiny all-reduce;
   BN-apply + LeakyReLU is a single scalar-engine Prelu op
   (out = prelu(rst*a + b, alpha)).
 - MLP head weights are staged into dead SBUF (layer-3 rst + a spare y slot)
   during bn3/pooling, making the head DMA-free; pooling runs as fp8
   DoubleRow matmuls against a one-hot graph matrix, directly feature-major.
"""

import math

import numpy as np
import ml_dtypes

BF = ml_dtypes.bfloat16
F8 = ml_dtypes.float8_e4m3
C = 8          # cores
P = 128        # partitions
EPS = 1e-5
SLOPE = 0.01
SW = 64.0      # fp8 weight pre-scale (cancelled exactly by BatchNorm)


# --------------------------------------------------------------------------
# Host-side preprocessing (index manipulation + dtype casts / layout only)
# --------------------------------------------------------------------------

def _tile_w(W, scale=1.0, dtype=BF):
    """[Kin, Mout] -> [128, Mout/128, Kin/128, 128] so that
    W_sb[p, fo, k, m] = W[k*128+p, fo*128+m] (lhsT k-tiles adjacent)."""
    Ki, Mo = W.shape
    return np.ascontiguousarray(
        (W * scale).reshape(Ki // P, P, Mo // P, P).transpose(1, 2, 0, 3)
    ).astype(dtype)


def _strip(v, ft):
    """[D] -> [128, D/128] fp32 with [p, t] = v[t*128+p]."""
    return np.ascontiguousarray(v.reshape(ft, P).T).astype(np.float32)


def _preprocess(inputs, G=64):
    h = np.asarray(inputs["h"], np.float32)
    src = np.asarray(inputs["src"], np.int64)
    dst = np.asarray(inputs["dst"], np.int64)
    graph_id = np.asarray(inputs["graph_id"], np.int64)
    N, IN_F = h.shape
    HID = np.asarray(inputs["Ws1"]).shape[1]
    MID = np.asarray(inputs["fc2_w"]).shape[1]
    NCLS = np.asarray(inputs["fc3_w"]).shape[1]
    Nc = N // C
    NG = Nc // P          # dst groups (of 128 nodes) per core
    NT = Nc // P          # node tiles per core
    FT = HID // P
    MT = MID // P

    # --- per-core edge partition, sorted by dst, grouped by 128-node groups
    per_core = []
    gmax = np.ones(NG, np.int64)
    for c in range(C):
        lo = c * Nc
        m = (dst >= lo) & (dst < lo + Nc)
        es = src[m]
        ed = dst[m] - lo
        order = np.argsort(ed, kind="stable")
        es, ed = es[order], ed[order]
        gcnt = np.bincount(ed // P, minlength=NG)
        gmax = np.maximum(gmax, gcnt)
        per_core.append((es, ed, gcnt))
    Kg = [int(x) for x in (gmax + P - 1) // P]
    K = max(Kg)
    IDXW = K * P // 16

    # --- gather indices + one-hot S matrices + 1/deg rows per core
    idx_all, S_all, pmat_all, invdeg_all = [], [], [], []
    for c in range(C):
        es, ed, gcnt = per_core[c]
        deg = np.bincount(ed, minlength=Nc).astype(np.float64)
        inv = (1.0 / np.maximum(deg, 1.0)).astype(np.float32)
        gstart = np.concatenate([[0], np.cumsum(gcnt)])
        idx16 = np.zeros((16, NG, IDXW), np.int16)
        S = np.zeros((P, NG, K, P), np.float32)   # [slot%128, g, slot//128, dst]
        for g in range(NG):
            seg_s = es[gstart[g]:gstart[g + 1]]
            seg_d = ed[gstart[g]:gstart[g + 1]] - g * P
            n = len(seg_s)
            j = np.arange(n)
            idx16[j % 16, g, j // 16] = seg_s.astype(np.int16)
            S[j % P, g, j // P, seg_d] = 1.0
        idx_all.append(np.tile(idx16, (8, 1, 1)))       # replicate for 8 Q7 cores
        S_all.append(S.astype(F8))
        invdeg_all.append(np.ascontiguousarray(
            np.tile(inv[None, :], (P, 1))).astype(BF))

        gid = graph_id[c * Nc:(c + 1) * Nc]
        pm = np.zeros((P, NT, G), np.float32)     # [node%128, node//128, graph]
        nn = np.arange(Nc)
        pm[nn % P, nn // P, gid] = 1.0
        pmat_all.append(pm.astype(F8))

    cnt = np.bincount(graph_id, minlength=G).astype(np.float64)
    invcnt = (1.0 / np.maximum(cnt, 1.0)).astype(np.float32)
    invcnt_bc = np.ascontiguousarray(np.tile(invcnt[None, :], (P, 1)))

    # --- feature tensors (fp8)
    h128 = np.zeros((N, 2 * P), np.float32)
    h128[:, :IN_F] = h
    h128 = h128.astype(F8)
    hT_all = []
    for c in range(C):
        ht = np.zeros((64, Nc), np.float32)
        ht[:IN_F] = h[c * Nc:(c + 1) * Nc].T
        hT_all.append(ht.astype(BF))

    # layer-1 combined weights: [64, 2, FT, 128], slot 0 = Ws1, slot 1 = Wn1
    # (bf16: layer 1 self path and weights stay full precision)
    w1 = np.zeros((64, 2, FT, P), np.float32)
    w1[:IN_F, 0] = np.asarray(inputs["Ws1"], np.float32).reshape(IN_F, FT, P)
    w1[:IN_F, 1] = np.asarray(inputs["Wn1"], np.float32).reshape(IN_F, FT, P)

    shared = {
        "h128": h128,
        "w1": w1.astype(BF),
        "w2s": _tile_w(np.asarray(inputs["Ws2"], np.float32), SW, F8),
        "w2n": _tile_w(np.asarray(inputs["Wn2"], np.float32), SW, F8),
        "w3s": _tile_w(np.asarray(inputs["Ws3"], np.float32), SW, F8),
        "w3n": _tile_w(np.asarray(inputs["Wn3"], np.float32), SW, F8),
        "wf1": _tile_w(np.asarray(inputs["fc1_w"], np.float32)),
        "wf2": _tile_w(np.asarray(inputs["fc2_w"], np.float32)),
        "wf3": np.ascontiguousarray(
            np.asarray(inputs["fc3_w"], np.float32).reshape(MT, P, NCLS)
            .transpose(1, 0, 2)).astype(BF),
        "bn1g": _strip(np.asarray(inputs["g1"], np.float32), FT),
        "bn1b": _strip(np.asarray(inputs["be1"], np.float32), FT),
        "bn2g": _strip(np.asarray(inputs["g2"], np.float32), FT),
        "bn2b": _strip(np.asarray(inputs["be2"], np.float32), FT),
        "bn3g": _strip(np.asarray(inputs["g3"], np.float32), FT),
        "bn3b": _strip(np.asarray(inputs["be3"], np.float32), FT),
        "f1b": _strip(np.asarray(inputs["fc1_b"], np.float32), FT),
        "f2b": _strip(np.asarray(inputs["fc2_b"], np.float32), MT),
        "f3b": np.asarray(inputs["fc3_b"], np.float32)[:, None].copy(),
        "invcnt": invcnt_bc,
        "chain": np.zeros((G, NCLS), np.float32),
    }
    in_maps = []
    for c in range(C):
        m = dict(shared)
        m.update({
            "hT": hT_all[c],
            "gidx": idx_all[c],
            "smat": S_all[c],
            "invdeg": invdeg_all[c],
            "pmat": pmat_all[c],
        })
        in_maps.append(m)

    meta = dict(N=N, Nc=Nc, NG=NG, NT=NT, FT=FT, MT=MT, HID=HID, MID=MID,
                NCLS=NCLS, K=K, IDXW=IDXW, G=G, Kg=Kg)
    return in_maps, meta


# --------------------------------------------------------------------------
# Bass program
# --------------------------------------------------------------------------

def _build(meta):
    import concourse.bass as bass
    import concourse.mybir as mybir
    import concourse.tile as tile
    from concourse import bacc
    from concourse.masks import make_identity

    dt = mybir.dt
    ALU = mybir.AluOpType
    ACT = mybir.ActivationFunctionType
    DR = mybir.MatmulPerfMode.DoubleRow

    N, Nc, NG, NT, FT, MT = (meta["N"], meta["Nc"], meta["NG"], meta["NT"],
                             meta["FT"], meta["MT"])
    HID, MID, NCLS = meta["HID"], meta["MID"], meta["NCLS"]
    K, IDXW, G = meta["K"], meta["IDXW"], meta["G"]
    Kg = meta["Kg"]
    NCH = 4                      # 512-node chunks per core
    CH = Nc // NCH

    import os
    NOCC = bool(os.environ.get("GCN_NOCC"))
    STAGE = os.environ.get("GCN_STAGE", "full")
    rg = [list(range(C))]

    nc = bacc.Bacc("TRN2", target_bir_lowering=False, debug=False,
                   num_devices=1 if NOCC else C)

    def collective(kind, op, ins, outs):
        if NOCC:
            iap, oap = ins[0], outs[0]
            if kind == "AllGather":
                nc.gpsimd.dma_start(oap[:iap.shape[0]], iap)
            else:
                nc.gpsimd.dma_start(oap, iap)
        else:
            nc.gpsimd.collective_compute(kind, op, replica_groups=rg,
                                         ins=[ins[0].opt()], outs=[outs[0].opt()])

    # ---- inputs
    t_h128 = nc.dram_tensor("h128", [N, 2 * P], dt.float8e4, kind="ExternalInput")
    t_hT = nc.dram_tensor("hT", [64, Nc], dt.bfloat16, kind="ExternalInput")
    t_gidx = nc.dram_tensor("gidx", [P, NG, IDXW], dt.int16, kind="ExternalInput")
    t_smat = nc.dram_tensor("smat", [P, NG, K, P], dt.float8e4, kind="ExternalInput")
    t_invdeg = nc.dram_tensor("invdeg", [P, Nc], dt.bfloat16, kind="ExternalInput")
    t_w1 = nc.dram_tensor("w1", [64, 2, FT, P], dt.bfloat16, kind="ExternalInput")
    t_w = {}
    for nm in ("w2s", "w2n", "w3s", "w3n"):
        t_w[nm] = nc.dram_tensor(nm, [P, FT, FT, P], dt.float8e4,
                                 kind="ExternalInput")
    t_w["wf1"] = nc.dram_tensor("wf1", [P, FT, FT, P], dt.bfloat16,
                                kind="ExternalInput")
    t_w["wf2"] = nc.dram_tensor("wf2", [P, MT, FT, P], dt.bfloat16,
                                kind="ExternalInput")
    t_wf3 = nc.dram_tensor("wf3", [P, MT, NCLS], dt.bfloat16, kind="ExternalInput")
    t_bn = {}
    for nm in ("bn1g", "bn1b", "bn2g", "bn2b", "bn3g", "bn3b", "f1b"):
        t_bn[nm] = nc.dram_tensor(nm, [P, FT], dt.float32, kind="ExternalInput")
    t_bn["f2b"] = nc.dram_tensor("f2b", [P, MT], dt.float32, kind="ExternalInput")
    t_f3b = nc.dram_tensor("f3b", [NCLS, 1], dt.float32, kind="ExternalInput")
    t_pmat = nc.dram_tensor("pmat", [P, NT, G], dt.float8e4, kind="ExternalInput")
    t_invcnt = nc.dram_tensor("invcnt", [P, G], dt.float32, kind="ExternalInput")
    t_out = nc.dram_tensor("out", [G, NCLS], dt.float32, kind="ExternalOutput")
    t_chain = nc.dram_tensor("chain", [G, NCLS], dt.float32, kind="ExternalInput")

    with tile.TileContext(nc) as tc:
        import contextlib
        ctx = contextlib.ExitStack()
        with ctx:
            dram = ctx.enter_context(tc.tile_pool(name="dram", bufs=1, space="DRAM"))
            consts = ctx.enter_context(tc.tile_pool(name="consts", bufs=1))
            work = ctx.enter_context(tc.tile_pool(name="work", bufs=1))
            psp = ctx.enter_context(tc.tile_pool(name="psp", bufs=8, space="PSUM"))

            # ---- DRAM scratch
            ynm = dram.tile([Nc, HID], dt.float8e4)
            if NOCC:
                yfull = [dram.tile([N, HID], dt.float8e4, name=f"yfull{i}")
                         for i in range(2)]
            else:
                yfull = [dram.tile([N, HID], dt.float8e4, addr_space="Shared",
                                   name=f"yfull{i}") for i in range(2)]
            stat_in = [dram.tile([P, 2 * FT], dt.float32, name=f"sti{i}")
                       for i in range(3)]
            stat_out = [dram.tile([P, 2 * FT], dt.float32, addr_space="Shared",
                                  name=f"sto{i}") for i in range(3)]
            pool_in = dram.tile([P, FT, G], dt.float32)
            pool_out = dram.tile([P, FT, G], dt.float32, addr_space="Shared")

            # ---- constants to SBUF
            idx_t = consts.tile([P, NG, IDXW], dt.int16)
            nc.sync.dma_start(idx_t[:], t_gidx[:])
            S_t = consts.tile([P, NG, K, P], dt.float8e4)
            nc.sync.dma_start(S_t[:], t_smat[:])
            w1_t = consts.tile([64, 2, FT, P], dt.bfloat16)
            nc.sync.dma_start(w1_t[:], t_w1[:])
            hm = consts.tile([64, 2, Nc], dt.bfloat16)
            nc.sync.dma_start(hm[:, 0, :], t_hT[:])
            invdeg_t = consts.tile([P, Nc], dt.bfloat16)
            nc.sync.dma_start(invdeg_t[:], t_invdeg[:])
            pmat_t = consts.tile([P, NT, G], dt.float8e4)
            nc.sync.dma_start(pmat_t[:], t_pmat[:])
            invcnt_t = consts.tile([P, G], dt.float32)
            nc.sync.dma_start(invcnt_t[:], t_invcnt[:])
            wf3_t = consts.tile([P, MT, NCLS], dt.bfloat16)
            nc.sync.dma_start(wf3_t[:], t_wf3[:])
            f3b_t = consts.tile([NCLS, 1], dt.float32)
            nc.sync.dma_start(f3b_t[:], t_f3b[:])
            bn_t = {}
            for nm, th in t_bn.items():
                bn_t[nm] = consts.tile(list(th.shape), dt.float32, name=f"c_{nm}")
                nc.sync.dma_start(bn_t[nm][:], th[:])
            ident_f8 = consts.tile([P, P], dt.float8e4)
            make_identity(nc, ident_f8[:])
            ident_f32 = consts.tile([32, 32], dt.float32)
            make_identity(nc, ident_f32[:])

            evac_ctr = [0]

            def evac(dst, src):
                """PSUM -> SBUF copy alternating between DVE and ACT."""
                evac_ctr[0] += 1
                if evac_ctr[0] % 2 == 0:
                    nc.vector.tensor_copy(dst, src)
                else:
                    nc.scalar.copy(dst, src)

            # ---------------- helpers ----------------
            def agg_group(li, m_fm, g):
                """Gather + m^T = (gathered y)^T @ S for one dst group
                (1/deg applied at evacuation; odd K tail uses a plain fp8
                matmul)."""
                ew = 2 * P if li == 1 else HID
                gsrc = t_h128 if li == 1 else yfull[li - 2]
                KG = Kg[g]
                KD, KT = KG // 2, KG % 2

                def chain(ps_reg, lhs_tile, fslice):
                    for k in range(KD):
                        nc.tensor.matmul(
                            ps_reg, lhsT=lhs_tile[:, 2 * k:2 * k + 2, fslice],
                            rhs=S_t[:, g, 2 * k:2 * k + 2, :],
                            start=(k == 0), stop=(KT == 0 and k == KD - 1),
                            perf_mode=DR, skip_group_check=True)
                    if KT:
                        nc.tensor.matmul(
                            ps_reg, lhsT=lhs_tile[:, KG - 1, fslice],
                            rhs=S_t[:, g, KG - 1, :],
                            start=(KD == 0), stop=True, skip_group_check=True)

                if li == 1:
                    Gt = work.tile([P, K, ew], dt.float8e4, tag="gt", bufs=2,
                                   name=f"G{li}_{g}")
                    nc.gpsimd.dma_gather(
                        out_ap=Gt[:, :KG, :], in_ap=gsrc[:],
                        idxs_ap=idx_t[:, g, :KG * 8],
                        num_idxs=KG * P, num_idxs_reg=KG * P, elem_size=ew)
                    ps = psp.tile([P, 512], dt.float32, tag="ps",
                                  name=f"aps{li}_{g}")
                    chain(ps[:64, :P], Gt, slice(0, 64))
                    nc.vector.tensor_tensor(
                        hm[:, 1, g * P:(g + 1) * P], ps[:64, :P],
                        invdeg_t[:64, g * P:(g + 1) * P], ALU.mult)
                    return
                HW = ew // 2
                for hh in range(2):
                    Gt = work.tile([P, K, HW], dt.float8e4, tag="gt",
                                   bufs=2, name=f"G{li}_{g}_{hh}")
                    nc.gpsimd.dma_gather(
                        out_ap=Gt[:, :KG, :],
                        in_ap=gsrc[:, hh * HW:(hh + 1) * HW],
                        idxs_ap=idx_t[:, g, :KG * 8],
                        num_idxs=KG * P, num_idxs_reg=KG * P,
                        elem_size=HW, elem_step=ew)
                    for ftg in range(2):
                        ps = psp.tile([P, 512], dt.float32, tag="ps",
                                      name=f"aps{li}_{g}_{hh}_{ftg}")
                        for j in range(4):
                            ft = ftg * 4 + j
                            chain(ps[:, j * P:(j + 1) * P], Gt,
                                  slice(ft * P, (ft + 1) * P))
                        nc.vector.tensor_tensor(
                            m_fm[:, hh * 8 + ftg * 4:hh * 8 + (ftg + 1) * 4,
                                 g * P:(g + 1) * P],
                            ps.rearrange("p (f n) -> p f n", f=4),
                            invdeg_t[:, g * P:(g + 1) * P]
                            .unsqueeze(1).broadcast_to([P, 4, P]),
                            ALU.mult)

            def dense_phase(li, y_prev, m_fm, rst, stats6, sc):
                """rst = (y W_s + m W_n) * SW for node chunks of super-chunk
                sc (fp8 DoubleRow, fused paths), bf16 rst SBUF-resident +
                per-tile bn_stats.  Weights are streamed per super-chunk;
                while dense(sc=0) runs, the next super-chunk's agg groups are
                emitted between fo steps so gathers stream continuously."""
                for fo in range(FT):
                    if sc == 0 and fo % 2 == 1:
                        agg_group(li, m_fm, NG // 2 + fo // 2)
                    if li > 1:
                        wsc = work.tile([P, FT, P], dt.float8e4, tag="wcol",
                                        bufs=3, name=f"ws{li}_{sc}_{fo}")
                        nc.sync.dma_start(wsc[:], t_w[f"w{li}s"][:, fo])
                        wnc = work.tile([P, FT, P], dt.float8e4, tag="wcol",
                                        bufs=3, name=f"wn{li}_{sc}_{fo}")
                        nc.sync.dma_start(wnc[:], t_w[f"w{li}n"][:, fo])
                    for ch in range(2 * sc, 2 * sc + 2):
                        sl = slice(ch * CH, (ch + 1) * CH)
                        ps = psp.tile([P, 512], dt.float32, tag="ps",
                                      name=f"dps{li}_{fo}_{ch}")
                        if li == 1:
                            for j in range(2):
                                nc.tensor.matmul(ps[:], lhsT=w1_t[:, j, fo, :],
                                                 rhs=hm[:, j, sl],
                                                 start=(j == 0), stop=(j == 1))
                        else:
                            for k in range(FT // 2):
                                nc.tensor.matmul(
                                    ps[:], lhsT=wsc[:, 2 * k:2 * k + 2, :],
                                    rhs=y_prev[:, 2 * k:2 * k + 2, sl],
                                    start=(k == 0), stop=False, perf_mode=DR)
                            for k in range(FT // 2):
                                nc.tensor.matmul(
                                    ps[:], lhsT=wnc[:, 2 * k:2 * k + 2, :],
                                    rhs=m_fm[:, 2 * k:2 * k + 2, sl],
                                    start=False, stop=(k == FT // 2 - 1),
                                    perf_mode=DR)
                        nc.scalar.copy(rst[:, fo, sl], ps[:])
                        nc.vector.bn_stats(stats6[:, fo, ch * 6:(ch + 1) * 6],
                                           rst[:, fo, sl])

            def stats_phase(li, stats6):
                """bn_aggr -> local (mu, var) -> AllReduce(sum, sumsq) -> a, b."""
                muvar = work.tile([P, FT, 2], dt.float32, tag="acc", bufs=3,
                                  name=f"mv{li}")
                for ft in range(FT):
                    nc.vector.bn_aggr(muvar[:, ft, :], stats6[:, ft, :])
                statio = work.tile([P, 2 * FT], dt.float32, tag="acc", bufs=3,
                                   name=f"sio{li}")
                # sum = mu*Nc ; sumsq = (var + mu^2)*Nc
                tmp = work.tile([P, FT], dt.float32, tag="acc2", bufs=3,
                                name=f"tmp{li}")
                nc.vector.tensor_scalar(statio[:, :FT], muvar[:, :, 0], float(Nc),
                                        None, ALU.mult)
                nc.vector.tensor_tensor(tmp[:], muvar[:, :, 0], muvar[:, :, 0],
                                        ALU.mult)
                nc.vector.tensor_tensor(tmp[:], muvar[:, :, 1], tmp[:], ALU.add)
                nc.vector.tensor_scalar(statio[:, FT:], tmp[:], float(Nc),
                                        None, ALU.mult)
                nc.gpsimd.dma_start(stat_in[li - 1][:], statio[:])
                collective("AllReduce", ALU.add, [stat_in[li - 1]],
                           [stat_out[li - 1]])
                sums = work.tile([P, 2 * FT], dt.float32, tag="sums", bufs=1,
                                 name=f"sm{li}")
                nc.gpsimd.dma_start(sums[:], stat_out[li - 1][:])
                mu = work.tile([P, FT], dt.float32, tag="acc", bufs=3,
                               name=f"mu{li}")
                var = work.tile([P, FT], dt.float32, tag="acc", bufs=3,
                                name=f"vr{li}")
                nc.vector.tensor_scalar(mu[:], sums[:, :FT], 1.0 / N, None,
                                        ALU.mult)
                nc.vector.tensor_scalar(var[:], sums[:, FT:], 1.0 / N, None,
                                        ALU.mult)
                tm2 = work.tile([P, FT], dt.float32, tag="acc2", bufs=3,
                                name=f"tm{li}")
                nc.vector.tensor_tensor(tm2[:], mu[:], mu[:], ALU.mult)
                nc.vector.tensor_tensor(var[:], var[:], tm2[:], ALU.subtract)
                nc.vector.tensor_scalar(var[:], var[:], EPS, None, ALU.add)
                std = work.tile([P, FT], dt.float32, tag="acc2", bufs=3,
                                name=f"sd{li}")
                nc.scalar.activation(std[:], var[:], ACT.Sqrt)
                rstd = work.tile([P, FT], dt.float32, tag="acc2", bufs=3,
                                 name=f"rs{li}")
                nc.vector.reciprocal(rstd[:], std[:])
                a_sb = work.tile([P, FT], dt.float32, tag="ab", bufs=2,
                                 name=f"a{li}")
                b_sb = work.tile([P, FT], dt.float32, tag="ab", bufs=2,
                                 name=f"b{li}")
                nc.vector.tensor_tensor(a_sb[:], rstd[:], bn_t[f"bn{li}g"][:],
                                        ALU.mult)
                nc.vector.tensor_tensor(tm2[:], mu[:], a_sb[:], ALU.mult)
                nc.vector.tensor_tensor(b_sb[:], bn_t[f"bn{li}b"][:], tm2[:],
                                        ALU.subtract)
                return a_sb, b_sb

            def bn_apply_phase(li, rst, a_sb, b_sb, y_new, y3t):
                """y = prelu(a*rst + b); transpose to node-major (fp8);
                li<3 -> ynm HBM, li==3 -> y3t SBUF."""
                for ft in range(FT):
                    nc.scalar.activation(y_new[:, ft, :], rst[:, ft, :],
                                         ACT.Prelu,
                                         bias=b_sb[:, ft:ft + 1],
                                         scale=a_sb[:, ft:ft + 1],
                                         alpha=SLOPE)
                for nt in range(NT):
                    if li < 3:
                        yT = work.tile([P, HID], dt.float8e4, tag="yT", bufs=2,
                                       name=f"yT{li}_{nt}")
                    for fh in range(2):
                        # fp8 transpose writes with element step 2 (hw rule)
                        tp = psp.tile([P, 2048], dt.float8e4, tag="ps",
                                      name=f"ytp{li}_{nt}_{fh}")
                        for j in range(8):
                            ft = fh * 8 + j
                            o = tp[:, j * 256:(j + 1) * 256].rearrange(
                                "p (n two) -> p n two", two=2)[:, :, 0]
                            nc.tensor.transpose(o,
                                                y_new[:, ft, nt * P:(nt + 1) * P],
                                                ident_f8[:])
                        dst = (yT[:, fh * 1024:(fh + 1) * 1024] if li < 3
                               else y3t[:, nt, fh * 1024:(fh + 1) * 1024])
                        src = tp.rearrange("p (blk n two) -> p blk n two",
                                           blk=8, two=2)[:, :, :, 0]
                        evac(dst, src)
                    if li < 3:
                        nc.gpsimd.dma_start(ynm[nt * P:(nt + 1) * P, :], yT[:])
                        # pipelined allgather substitute: publish the local
                        # slice of this node tile immediately
                        if NOCC:
                            nc.gpsimd.dma_start(
                                yfull[li - 1][nt * P:(nt + 1) * P, :],
                                ynm[nt * P:(nt + 1) * P, :])

            # ---------------- the network ----------------
            ym = {}
            def ym_tile(name):
                return work.tile([P, FT, Nc], dt.float8e4, tag="ym", bufs=2,
                                 name=name)

            y3t = None
            done = False
            for li in (1, 2, 3):
                stats6 = work.tile([P, FT, NCH * 6], dt.float32, tag="st6",
                                   bufs=1, name=f"st6_{li}")
                rst = work.tile([P, FT, Nc], dt.bfloat16, tag="rst", bufs=1,
                                name=f"rst{li}")
                m_fm = None if li == 1 else ym_tile(f"m{li}")
                for g in range(NG // 2):
                    agg_group(li, m_fm, g)
                if STAGE == f"agg{li}":
                    done = True
                    break
                dense_phase(li, ym.get("y"), m_fm, rst, stats6, 0)
                dense_phase(li, ym.get("y"), m_fm, rst, stats6, 1)
                if STAGE == f"dense{li}":
                    done = True
                    break
                a_sb, b_sb = stats_phase(li, stats6)
                y_new = ym_tile(f"y{li}")
                if li == 3:
                    y3t = ym_tile("y3t")
                bn_apply_phase(li, rst, a_sb, b_sb, y_new, y3t)
                ym["y"] = y_new
                if STAGE == f"bn{li}":
                    done = True
                    break
                if li < 3 and not NOCC:
                    collective("AllGather", ALU.bypass, [ynm], [yfull[li - 1]])

            if STAGE == "full" and not done:
                # stage MLP head weights into dead SBUF (rst of layer 3 and a
                # spare ym slot) while bn3/pooling run, so the head is DMA-free
                for fo in range(FT):
                    nc.sync.dma_start(rst[:, fo, :], t_w["wf1"][:, fo])
                yw = ym_tile("yw").bitcast(dt.bfloat16)   # [P, FT, Nc//2] bf16
                for fo in range(MT):
                    nc.sync.dma_start(yw[:, 2 * fo:2 * fo + 2, :1024],
                                      t_w["wf2"][:, fo])

                def wf1_col(fo):
                    return rst[:, fo, :].rearrange("p (k m) -> p k m", k=FT)

                def wf2_col(fo):
                    return yw[:, 2 * fo:2 * fo + 2, :1024].rearrange(
                        "p a (k m) -> p (a k) m", k=FT // 2)

                # ---------------- pooling (feature-major) ----------------
                pps = [psp.tile([P, 512], dt.float32, tag="ps", name=f"pps{j}")
                       for j in range(2)]
                for ft in range(FT):
                    reg = pps[ft // 8][:, (ft % 8) * G:(ft % 8 + 1) * G]
                    for i in range(NT // 2):
                        nc.tensor.matmul(
                            reg,
                            lhsT=y3t[:, 2 * i:2 * i + 2, ft * P:(ft + 1) * P],
                            rhs=pmat_t[:, 2 * i:2 * i + 2, :],
                            start=(i == 0), stop=(i == NT // 2 - 1),
                            perf_mode=DR, skip_group_check=True)
                pool_sb = work.tile([P, FT, G], dt.float32, tag="pool", bufs=1)
                for j in range(2):
                    evac(pool_sb[:, j * 8:(j + 1) * 8, :], pps[j][:])
                nc.gpsimd.dma_start(pool_in[:], pool_sb[:])
                collective("AllReduce", ALU.add, [pool_in], [pool_out])
                hgsum = work.tile([P, FT, G], dt.float32, tag="pool2", bufs=1)
                nc.gpsimd.dma_start(hgsum[:], pool_out[:])
                hg_bf = work.tile([P, FT, G], dt.bfloat16, tag="hg", bufs=1)
                for ft in range(FT):
                    nc.vector.tensor_tensor(hg_bf[:, ft, :], hgsum[:, ft, :],
                                            invcnt_t[:], ALU.mult)

                # ---------------- MLP head (bf16) ----------------
                def fc_layer(wcol_fn, kt_count, fo_count, xin, bias_t, name):
                    xout = work.tile([P, fo_count, G], dt.bfloat16,
                                     tag=f"x{name}", bufs=1, name=f"x{name}")
                    for fo in range(fo_count):
                        wc = wcol_fn(fo)
                        ps = psp.tile([P, 512], dt.float32, tag="ps",
                                      name=f"hps{name}_{fo}")
                        for k in range(kt_count):
                            nc.tensor.matmul(ps[:, :G], lhsT=wc[:, k, :],
                                             rhs=xin[:, k, :],
                                             start=(k == 0),
                                             stop=(k == kt_count - 1))
                        nc.scalar.activation(xout[:, fo, :], ps[:, :G],
                                             ACT.Prelu,
                                             bias=bias_t[:, fo:fo + 1],
                                             scale=1.0, alpha=SLOPE)
                    return xout

                x1 = fc_layer(wf1_col, FT, FT, hg_bf, bn_t["f1b"], "f1")
                x2 = fc_layer(wf2_col, FT, MT, x1, bn_t["f2b"], "f2")

                ps18 = psp.tile([P, 512], dt.float32, tag="ps", name="ps18")
                for k in range(MT):
                    nc.tensor.matmul(ps18[:NCLS, :G], lhsT=wf3_t[:, k, :],
                                     rhs=x2[:, k, :], start=(k == 0),
                                     stop=(k == MT - 1))
                o18 = work.tile([NCLS, G], dt.float32, tag="o18", bufs=1)
                nc.vector.tensor_scalar(o18[:], ps18[:NCLS, :G], f3b_t[:, 0:1],
                                        None, ALU.add)
                tp = psp.tile([P, 256], dt.bfloat16, tag="ps", name="otp")
                tpf = tp.bitcast(dt.float32)
                nc.tensor.transpose(tpf[:G, :NCLS], o18[:], ident_f32[:NCLS, :NCLS])
                osb = work.tile([G, NCLS], dt.float32, tag="osb", bufs=1)
                nc.vector.tensor_copy(osb[:], tpf[:G, :NCLS])
                chn = work.tile([G, NCLS], dt.float32, tag="chn", bufs=1)
                nc.sync.dma_start(chn[:], t_chain[:])
                nc.vector.tensor_scalar(chn[:], chn[:], 0.0, None, ALU.mult)
                nc.vector.tensor_tensor(osb[:], osb[:], chn[:], ALU.add)
                nc.sync.dma_start(t_out[:], osb[:])
            else:
                # debug early-exit: emit *something* into out
                dbg = work.tile([G, NCLS], dt.float32, tag="dbg", bufs=1)
                nc.vector.memset(dbg[:], 0.0)
                nc.sync.dma_start(t_out[:], dbg[:])

    nc.compile()
    return nc


# --------------------------------------------------------------------------
# entry point
# --------------------------------------------------------------------------

LAST_EXEC_NS = None
LAST_TRACE = None


def _run_timed(nc, in_maps, iters=4, reps=None):
    """Mirror bass2jax.run_bass_via_pjrt but keep inputs device-resident so
    warm re-executions measure the on-device program span."""
    import time
    import jax
    import jax.numpy as jnp
    from jax.sharding import Mesh, PartitionSpec
    from jax.experimental.shard_map import shard_map
    import concourse.mybir as mybir
    from concourse.bass2jax import (
        install_neuronx_cc_hook, _bass_exec_p, partition_id_tensor)

    install_neuronx_cc_hook()
    n_cores = len(in_maps)
    partition_name = nc.partition_id_tensor.name if nc.partition_id_tensor else None
    in_names, out_names, out_avals, zero_outs = [], [], [], []
    for alloc in nc.m.functions[0].allocations:
        if not isinstance(alloc, mybir.MemoryLocationSet):
            continue
        name = alloc.memorylocations[0].name
        if alloc.kind == "ExternalInput":
            if name != partition_name:
                in_names.append(name)
        elif alloc.kind == "ExternalOutput":
            shape = tuple(alloc.tensor_shape)
            dtype = mybir.dt.np(alloc.dtype)
            out_names.append(name)
            out_avals.append(jax.core.ShapedArray(shape, dtype))
            zero_outs.append(np.zeros((n_cores * shape[0], *shape[1:]), dtype))
    n_params = len(in_names)
    all_in = list(in_names) + list(out_names)
    if partition_name is not None:
        all_in.append(partition_name)

    import os
    if reps is None:
        reps = int(os.environ.get("GCN_REPS", "1"))

    chain_idx = in_names.index("chain") if "chain" in in_names else None
    out_idx = out_names.index("out") if "out" in out_names else None

    def _body(*args):
        operands = list(args)
        if partition_name is not None:
            operands.append(partition_id_tensor())
        for _ in range(reps):
            outs = _bass_exec_p.bind(
                *operands, out_avals=tuple(out_avals), in_names=tuple(all_in),
                out_names=tuple(out_names), lowering_input_output_aliases=(),
                sim_require_finite=True, sim_require_nnan=True, nc=nc)
            if chain_idx is not None and out_idx is not None:
                operands[chain_idx] = outs[out_idx]
        return tuple(outs)

    devices = jax.devices()[:n_cores]
    mesh = Mesh(np.asarray(devices), ("core",))
    nin = n_params + len(out_names)
    sharded = jax.jit(
        shard_map(_body, mesh=mesh, in_specs=(PartitionSpec("core"),) * nin,
                  out_specs=(PartitionSpec("core"),) * len(out_names),
                  check_rep=False),
        donate_argnums=tuple(range(n_params, nin)), keep_unused=True)

    shd = jax.sharding.NamedSharding(mesh, PartitionSpec("core"))
    dev_in = [
        jax.device_put(
            np.concatenate([np.asarray(in_maps[c][nm]) for c in range(n_cores)],
                           axis=0), shd)
        for nm in in_names
    ]
    times = []
    outs = None
    for _ in range(iters):
        zo = [jax.device_put(z.copy(), shd) for z in zero_outs]
        for z in zo:
            z.block_until_ready()
        t0 = time.perf_counter()
        outs = sharded(*dev_in, *zo)
        for o in outs:
            o.block_until_ready()
        times.append(time.perf_counter() - t0)
    best_ns = int(min(times) * 1e9 / reps)
    results = [
        {nm: np.asarray(outs[i]).reshape(n_cores, *out_avals[i].shape)[c]
         for i, nm in enumerate(out_names)}
        for c in range(n_cores)
    ]
    print(f"timed runs (s, reps={reps}): {[f'{t:.4f}' for t in times]}")
    return results, best_ns


def kernel(**inputs) -> np.ndarray:
    global LAST_EXEC_NS, LAST_TRACE
    from concourse.bass_utils import run_bass_kernel_spmd

    import os

    in_maps, meta = _preprocess(inputs)
    nc = _build(meta)
    in_maps = [{k: np.ascontiguousarray(v) for k, v in m.items()}
               for m in in_maps]
    if os.environ.get("GCN_TIME"):
        results, best_ns = _run_timed(nc, in_maps)
        LAST_EXEC_NS = best_ns
        return np.asarray(results[0]["out"], np.float32)
    res = run_bass_kernel_spmd(nc, in_maps, core_ids=list(range(C)))
    LAST_EXEC_NS = res.exec_time_ns
    LAST_TRACE = res.instructions_and_trace
    return np.asarray(res.results[0]["out"], np.float32)


# revision 30
# speedup vs baseline: 1.0545x; 1.0545x over previous
"""GraphSAGE (3x SAGEConv-mean + BN + LeakyReLU) + AvgPool + MLP head on 8 Trainium2
NeuronCores via Bass/Tile.

Sharding: nodes are partitioned contiguously across the 8 cores (2048 each);
weights are replicated; BatchNorm statistics and per-graph pooled sums are
all-reduced; per-layer activations are all-gathered (node-major fp8 in HBM)
so each core can gather the source rows of its incident edges.

Fast path vs the bf16 baseline:
 - Layer-2/3 matmuls (self+neighbor fused into one PSUM accumulation) run in
   fp8e4 with MatmulPerfMode.DoubleRow (2 k-tiles per pass).  BatchNorm makes
   each layer scale-invariant, so weights are pre-scaled by 64 on the host
   for fp8 range with no descale anywhere.  Layer 1 (raw input) stays bf16
   for accuracy; its neighbor mean is fp8.
 - Neighbor aggregation computes m^T = (gathered)^T @ S directly
   (feature-major output, no aggregation transposes); S is an exact one-hot
   fp8 matrix and 1/deg is applied in the PSUM evacuation as an exact bf16
   broadcast multiply.
 - y and m stay SBUF-resident feature-major (no sp/m/rst HBM roundtrips, no
   DMA transposes); node-major fp8 y tiles (stride-2 fp8 PE transposes) are
   written to HBM only for the gather/allgather.
 - Each layer's dense fo-loop is emission-interleaved with the next
   super-chunk's gathers/aggregation so PE, DMA and GPSIMD stream
   continuously; dst-group chunk counts are odd-capable (plain fp8 tail
   matmul) to avoid gather padding.
 - BatchNorm statistics via DVE bn_stats/bn_aggr + a tiny all-reduce;
   BN-apply + LeakyReLU is a single scalar-engine Prelu op
   (out = prelu(rst*a + b, alpha)).
 - MLP head weights are staged into dead SBUF (layer-3 rst + a spare y slot)
   during bn3/pooling, making the head DMA-free; pooling runs as fp8
   DoubleRow matmuls against a one-hot graph matrix, directly feature-major.
"""

import math

import numpy as np
import ml_dtypes

BF = ml_dtypes.bfloat16
F8 = ml_dtypes.float8_e4m3
C = 8          # cores
P = 128        # partitions
EPS = 1e-5
SLOPE = 0.01
SW = 64.0      # fp8 weight pre-scale (cancelled exactly by BatchNorm)


# --------------------------------------------------------------------------
# Host-side preprocessing (index manipulation + dtype casts / layout only)
# --------------------------------------------------------------------------

def _tile_w(W, scale=1.0, dtype=BF):
    """[Kin, Mout] -> [128, Mout/128, Kin/128, 128] so that
    W_sb[p, fo, k, m] = W[k*128+p, fo*128+m] (lhsT k-tiles adjacent)."""
    Ki, Mo = W.shape
    return np.ascontiguousarray(
        (W * scale).reshape(Ki // P, P, Mo // P, P).transpose(1, 2, 0, 3)
    ).astype(dtype)


def _strip(v, ft):
    """[D] -> [128, D/128] fp32 with [p, t] = v[t*128+p]."""
    return np.ascontiguousarray(v.reshape(ft, P).T).astype(np.float32)


def _preprocess(inputs, G=64):
    h = np.asarray(inputs["h"], np.float32)
    src = np.asarray(inputs["src"], np.int64)
    dst = np.asarray(inputs["dst"], np.int64)
    graph_id = np.asarray(inputs["graph_id"], np.int64)
    N, IN_F = h.shape
    HID = np.asarray(inputs["Ws1"]).shape[1]
    MID = np.asarray(inputs["fc2_w"]).shape[1]
    NCLS = np.asarray(inputs["fc3_w"]).shape[1]
    Nc = N // C
    NG = Nc // P          # dst groups (of 128 nodes) per core
    NT = Nc // P          # node tiles per core
    FT = HID // P
    MT = MID // P

    # --- per-core edge partition, sorted by dst, grouped by 128-node groups
    per_core = []
    gmax = np.ones(NG, np.int64)
    for c in range(C):
        lo = c * Nc
        m = (dst >= lo) & (dst < lo + Nc)
        es = src[m]
        ed = dst[m] - lo
        order = np.argsort(ed, kind="stable")
        es, ed = es[order], ed[order]
        gcnt = np.bincount(ed // P, minlength=NG)
        gmax = np.maximum(gmax, gcnt)
        per_core.append((es, ed, gcnt))
    Kg = [int(x) for x in (gmax + P - 1) // P]
    K = max(Kg)
    IDXW = K * P // 16

    # --- gather indices + one-hot S matrices + 1/deg rows per core
    idx_all, S_all, pmat_all, invdeg_all = [], [], [], []
    for c in range(C):
        es, ed, gcnt = per_core[c]
        deg = np.bincount(ed, minlength=Nc).astype(np.float64)
        inv = (1.0 / np.maximum(deg, 1.0)).astype(np.float32)
        gstart = np.concatenate([[0], np.cumsum(gcnt)])
        idx16 = np.zeros((16, NG, IDXW), np.int16)
        S = np.zeros((P, NG, K, P), np.float32)   # [slot%128, g, slot//128, dst]
        for g in range(NG):
            seg_s = es[gstart[g]:gstart[g + 1]]
            seg_d = ed[gstart[g]:gstart[g + 1]] - g * P
            n = len(seg_s)
            j = np.arange(n)
            idx16[j % 16, g, j // 16] = seg_s.astype(np.int16)
            S[j % P, g, j // P, seg_d] = 1.0
        idx_all.append(np.tile(idx16, (8, 1, 1)))       # replicate for 8 Q7 cores
        S_all.append(S.astype(F8))
        invdeg_all.append(np.ascontiguousarray(
            np.tile(inv[None, :], (P, 1))).astype(BF))

        gid = graph_id[c * Nc:(c + 1) * Nc]
        pm = np.zeros((P, NT, G), np.float32)     # [node%128, node//128, graph]
        nn = np.arange(Nc)
        pm[nn % P, nn // P, gid] = 1.0
        pmat_all.append(pm.astype(F8))

    cnt = np.bincount(graph_id, minlength=G).astype(np.float64)
    invcnt = (1.0 / np.maximum(cnt, 1.0)).astype(np.float32)
    invcnt_bc = np.ascontiguousarray(np.tile(invcnt[None, :], (P, 1)))

    # --- feature tensors (fp8)
    h128 = np.zeros((N, 2 * P), np.float32)
    h128[:, :IN_F] = h
    h128 = h128.astype(F8)
    hT_all = []
    for c in range(C):
        ht = np.zeros((64, Nc), np.float32)
        ht[:IN_F] = h[c * Nc:(c + 1) * Nc].T
        hT_all.append(ht.astype(BF))

    # layer-1 combined weights: [64, 2, FT, 128], slot 0 = Ws1, slot 1 = Wn1
    # (bf16: layer 1 self path and weights stay full precision)
    w1 = np.zeros((64, 2, FT, P), np.float32)
    w1[:IN_F, 0] = np.asarray(inputs["Ws1"], np.float32).reshape(IN_F, FT, P)
    w1[:IN_F, 1] = np.asarray(inputs["Wn1"], np.float32).reshape(IN_F, FT, P)

    shared = {
        "h128": h128,
        "w1": w1.astype(BF),
        "w2s": _tile_w(np.asarray(inputs["Ws2"], np.float32), SW, F8),
        "w2n": _tile_w(np.asarray(inputs["Wn2"], np.float32), SW, F8),
        "w3s": _tile_w(np.asarray(inputs["Ws3"], np.float32), SW, F8),
        "w3n": _tile_w(np.asarray(inputs["Wn3"], np.float32), SW, F8),
        "wf1": _tile_w(np.asarray(inputs["fc1_w"], np.float32)),
        "wf2": _tile_w(np.asarray(inputs["fc2_w"], np.float32)),
        "wf3": np.ascontiguousarray(
            np.asarray(inputs["fc3_w"], np.float32).reshape(MT, P, NCLS)
            .transpose(1, 0, 2)).astype(BF),
        "bn1g": _strip(np.asarray(inputs["g1"], np.float32), FT),
        "bn1b": _strip(np.asarray(inputs["be1"], np.float32), FT),
        "bn2g": _strip(np.asarray(inputs["g2"], np.float32), FT),
        "bn2b": _strip(np.asarray(inputs["be2"], np.float32), FT),
        "bn3g": _strip(np.asarray(inputs["g3"], np.float32), FT),
        "bn3b": _strip(np.asarray(inputs["be3"], np.float32), FT),
        "f1b": _strip(np.asarray(inputs["fc1_b"], np.float32), FT),
        "f2b": _strip(np.asarray(inputs["fc2_b"], np.float32), MT),
        "f3b": np.asarray(inputs["fc3_b"], np.float32)[:, None].copy(),
        "invcnt": invcnt_bc,
        "chain": np.zeros((G, NCLS), np.float32),
    }
    in_maps = []
    for c in range(C):
        m = dict(shared)
        m.update({
            "hT": hT_all[c],
            "gidx": idx_all[c],
            "smat": S_all[c],
            "invdeg": invdeg_all[c],
            "pmat": pmat_all[c],
        })
        in_maps.append(m)

    meta = dict(N=N, Nc=Nc, NG=NG, NT=NT, FT=FT, MT=MT, HID=HID, MID=MID,
                NCLS=NCLS, K=K, IDXW=IDXW, G=G, Kg=Kg)
    return in_maps, meta


# --------------------------------------------------------------------------
# Bass program
# --------------------------------------------------------------------------

def _build(meta):
    import concourse.bass as bass
    import concourse.mybir as mybir
    import concourse.tile as tile
    from concourse import bacc
    from concourse.masks import make_identity

    dt = mybir.dt
    ALU = mybir.AluOpType
    ACT = mybir.ActivationFunctionType
    DR = mybir.MatmulPerfMode.DoubleRow

    N, Nc, NG, NT, FT, MT = (meta["N"], meta["Nc"], meta["NG"], meta["NT"],
                             meta["FT"], meta["MT"])
    HID, MID, NCLS = meta["HID"], meta["MID"], meta["NCLS"]
    K, IDXW, G = meta["K"], meta["IDXW"], meta["G"]
    Kg = meta["Kg"]
    NCH = 4                      # 512-node chunks per core
    CH = Nc // NCH

    import os
    NOCC = bool(os.environ.get("GCN_NOCC"))
    STAGE = os.environ.get("GCN_STAGE", "full")
    rg = [list(range(C))]

    nc = bacc.Bacc("TRN2", target_bir_lowering=False, debug=False,
                   num_devices=1 if NOCC else C)

    def collective(kind, op, ins, outs):
        if NOCC:
            iap, oap = ins[0], outs[0]
            if kind == "AllGather":
                nc.gpsimd.dma_start(oap[:iap.shape[0]], iap)
            else:
                nc.gpsimd.dma_start(oap, iap)
        else:
            nc.gpsimd.collective_compute(kind, op, replica_groups=rg,
                                         ins=[ins[0].opt()], outs=[outs[0].opt()])

    # ---- inputs
    t_h128 = nc.dram_tensor("h128", [N, 2 * P], dt.float8e4, kind="ExternalInput")
    t_hT = nc.dram_tensor("hT", [64, Nc], dt.bfloat16, kind="ExternalInput")
    t_gidx = nc.dram_tensor("gidx", [P, NG, IDXW], dt.int16, kind="ExternalInput")
    t_smat = nc.dram_tensor("smat", [P, NG, K, P], dt.float8e4, kind="ExternalInput")
    t_invdeg = nc.dram_tensor("invdeg", [P, Nc], dt.bfloat16, kind="ExternalInput")
    t_w1 = nc.dram_tensor("w1", [64, 2, FT, P], dt.bfloat16, kind="ExternalInput")
    t_w = {}
    for nm in ("w2s", "w2n", "w3s", "w3n"):
        t_w[nm] = nc.dram_tensor(nm, [P, FT, FT, P], dt.float8e4,
                                 kind="ExternalInput")
    t_w["wf1"] = nc.dram_tensor("wf1", [P, FT, FT, P], dt.bfloat16,
                                kind="ExternalInput")
    t_w["wf2"] = nc.dram_tensor("wf2", [P, MT, FT, P], dt.bfloat16,
                                kind="ExternalInput")
    t_wf3 = nc.dram_tensor("wf3", [P, MT, NCLS], dt.bfloat16, kind="ExternalInput")
    t_bn = {}
    for nm in ("bn1g", "bn1b", "bn2g", "bn2b", "bn3g", "bn3b", "f1b"):
        t_bn[nm] = nc.dram_tensor(nm, [P, FT], dt.float32, kind="ExternalInput")
    t_bn["f2b"] = nc.dram_tensor("f2b", [P, MT], dt.float32, kind="ExternalInput")
    t_f3b = nc.dram_tensor("f3b", [NCLS, 1], dt.float32, kind="ExternalInput")
    t_pmat = nc.dram_tensor("pmat", [P, NT, G], dt.float8e4, kind="ExternalInput")
    t_invcnt = nc.dram_tensor("invcnt", [P, G], dt.float32, kind="ExternalInput")
    t_out = nc.dram_tensor("out", [G, NCLS], dt.float32, kind="ExternalOutput")
    t_chain = nc.dram_tensor("chain", [G, NCLS], dt.float32, kind="ExternalInput")

    with tile.TileContext(nc) as tc:
        import contextlib
        ctx = contextlib.ExitStack()
        with ctx:
            dram = ctx.enter_context(tc.tile_pool(name="dram", bufs=1, space="DRAM"))
            consts = ctx.enter_context(tc.tile_pool(name="consts", bufs=1))
            work = ctx.enter_context(tc.tile_pool(name="work", bufs=1))
            psp = ctx.enter_context(tc.tile_pool(name="psp", bufs=8, space="PSUM"))

            # ---- DRAM scratch
            ynm = dram.tile([Nc, HID], dt.float8e4)
            if NOCC:
                yfull = [dram.tile([N, HID], dt.float8e4, name=f"yfull{i}")
                         for i in range(2)]
            else:
                yfull = [dram.tile([N, HID], dt.float8e4, addr_space="Shared",
                                   name=f"yfull{i}") for i in range(2)]
            stat_in = [dram.tile([P, 2 * FT], dt.float32, name=f"sti{i}")
                       for i in range(3)]
            stat_out = [dram.tile([P, 2 * FT], dt.float32, addr_space="Shared",
                                  name=f"sto{i}") for i in range(3)]
            pool_in = dram.tile([P, FT, G], dt.float32)
            pool_out = dram.tile([P, FT, G], dt.float32, addr_space="Shared")

            # ---- constants to SBUF
            idx_t = consts.tile([P, NG, IDXW], dt.int16)
            nc.sync.dma_start(idx_t[:], t_gidx[:])
            S_t = consts.tile([P, NG, K, P], dt.float8e4)
            nc.sync.dma_start(S_t[:], t_smat[:])
            w1_t = consts.tile([64, 2, FT, P], dt.bfloat16)
            nc.sync.dma_start(w1_t[:], t_w1[:])
            hm = consts.tile([64, 2, Nc], dt.bfloat16)
            nc.sync.dma_start(hm[:, 0, :], t_hT[:])
            invdeg_t = consts.tile([P, Nc], dt.bfloat16)
            nc.sync.dma_start(invdeg_t[:], t_invdeg[:])
            pmat_t = consts.tile([P, NT, G], dt.float8e4)
            nc.sync.dma_start(pmat_t[:], t_pmat[:])
            invcnt_t = consts.tile([P, G], dt.float32)
            nc.sync.dma_start(invcnt_t[:], t_invcnt[:])
            wf3_t = consts.tile([P, MT, NCLS], dt.bfloat16)
            nc.sync.dma_start(wf3_t[:], t_wf3[:])
            f3b_t = consts.tile([NCLS, 1], dt.float32)
            nc.sync.dma_start(f3b_t[:], t_f3b[:])
            bn_t = {}
            for nm, th in t_bn.items():
                bn_t[nm] = consts.tile(list(th.shape), dt.float32, name=f"c_{nm}")
                nc.sync.dma_start(bn_t[nm][:], th[:])
            ident_f8 = consts.tile([P, P], dt.float8e4)
            make_identity(nc, ident_f8[:])
            ident_f32 = consts.tile([32, 32], dt.float32)
            make_identity(nc, ident_f32[:])

            evac_ctr = [0]

            def evac(dst, src):
                """PSUM -> SBUF copy alternating between DVE and ACT."""
                evac_ctr[0] += 1
                if evac_ctr[0] % 2 == 0:
                    nc.vector.tensor_copy(dst, src)
                else:
                    nc.scalar.copy(dst, src)

            # ---------------- helpers ----------------
            def agg_group(li, m_fm, g):
                """Gather + m^T = (gathered y)^T @ S for one dst group
                (1/deg applied at evacuation; odd K tail uses a plain fp8
                matmul)."""
                ew = 2 * P if li == 1 else HID
                gsrc = t_h128 if li == 1 else yfull[li - 2]
                KG = Kg[g]
                KD, KT = KG // 2, KG % 2

                def chain(ps_reg, lhs_tile, fslice):
                    for k in range(KD):
                        nc.tensor.matmul(
                            ps_reg, lhsT=lhs_tile[:, 2 * k:2 * k + 2, fslice],
                            rhs=S_t[:, g, 2 * k:2 * k + 2, :],
                            start=(k == 0), stop=(KT == 0 and k == KD - 1),
                            perf_mode=DR, skip_group_check=True)
                    if KT:
                        nc.tensor.matmul(
                            ps_reg, lhsT=lhs_tile[:, KG - 1, fslice],
                            rhs=S_t[:, g, KG - 1, :],
                            start=(KD == 0), stop=True, skip_group_check=True)

                if li == 1:
                    Gt = work.tile([P, K, ew], dt.float8e4, tag="gt", bufs=4,
                                   name=f"G{li}_{g}")
                    nc.gpsimd.dma_gather(
                        out_ap=Gt[:, :KG, :], in_ap=gsrc[:],
                        idxs_ap=idx_t[:, g, :KG * 8],
                        num_idxs=KG * P, num_idxs_reg=KG * P, elem_size=ew)
                    ps = psp.tile([P, 512], dt.float32, tag="ps",
                                  name=f"aps{li}_{g}")
                    chain(ps[:64, :P], Gt, slice(0, 64))
                    nc.vector.tensor_tensor(
                        hm[:, 1, g * P:(g + 1) * P], ps[:64, :P],
                        invdeg_t[:64, g * P:(g + 1) * P], ALU.mult)
                    return
                HW = ew // 2
                for hh in range(2):
                    Gt = work.tile([P, K, HW], dt.float8e4, tag="gt",
                                   bufs=4, name=f"G{li}_{g}_{hh}")
                    nc.gpsimd.dma_gather(
                        out_ap=Gt[:, :KG, :],
                        in_ap=gsrc[:, hh * HW:(hh + 1) * HW],
                        idxs_ap=idx_t[:, g, :KG * 8],
                        num_idxs=KG * P, num_idxs_reg=KG * P,
                        elem_size=HW, elem_step=ew)
                    for ftg in range(2):
                        ps = psp.tile([P, 512], dt.float32, tag="ps",
                                      name=f"aps{li}_{g}_{hh}_{ftg}")
                        for j in range(4):
                            ft = ftg * 4 + j
                            chain(ps[:, j * P:(j + 1) * P], Gt,
                                  slice(ft * P, (ft + 1) * P))
                        nc.vector.tensor_tensor(
                            m_fm[:, hh * 8 + ftg * 4:hh * 8 + (ftg + 1) * 4,
                                 g * P:(g + 1) * P],
                            ps.rearrange("p (f n) -> p f n", f=4),
                            invdeg_t[:, g * P:(g + 1) * P]
                            .unsqueeze(1).broadcast_to([P, 4, P]),
                            ALU.mult)

            def dense_phase(li, y_prev, m_fm, rst, stats6, sc):
                """rst = (y W_s + m W_n) * SW for node chunks of super-chunk
                sc (fp8 DoubleRow, fused paths), bf16 rst SBUF-resident +
                per-tile bn_stats.  Weights are streamed per super-chunk;
                while dense(sc=0) runs, the next super-chunk's agg groups are
                emitted between fo steps so gathers stream continuously."""
                for fo in range(FT):
                    if sc == 0 and fo % 2 == 1:
                        agg_group(li, m_fm, NG // 2 + fo // 2)
                    if li > 1:
                        wsc = work.tile([P, FT, P], dt.float8e4, tag="wcol",
                                        bufs=3, name=f"ws{li}_{sc}_{fo}")
                        nc.sync.dma_start(wsc[:], t_w[f"w{li}s"][:, fo])
                        wnc = work.tile([P, FT, P], dt.float8e4, tag="wcol",
                                        bufs=3, name=f"wn{li}_{sc}_{fo}")
                        nc.sync.dma_start(wnc[:], t_w[f"w{li}n"][:, fo])
                    for ch in range(2 * sc, 2 * sc + 2):
                        sl = slice(ch * CH, (ch + 1) * CH)
                        ps = psp.tile([P, 512], dt.float32, tag="ps",
                                      name=f"dps{li}_{fo}_{ch}")
                        if li == 1:
                            for j in range(2):
                                nc.tensor.matmul(ps[:], lhsT=w1_t[:, j, fo, :],
                                                 rhs=hm[:, j, sl],
                                                 start=(j == 0), stop=(j == 1))
                        else:
                            for k in range(FT // 2):
                                nc.tensor.matmul(
                                    ps[:], lhsT=wsc[:, 2 * k:2 * k + 2, :],
                                    rhs=y_prev[:, 2 * k:2 * k + 2, sl],
                                    start=(k == 0), stop=False, perf_mode=DR)
                            for k in range(FT // 2):
                                nc.tensor.matmul(
                                    ps[:], lhsT=wnc[:, 2 * k:2 * k + 2, :],
                                    rhs=m_fm[:, 2 * k:2 * k + 2, sl],
                                    start=False, stop=(k == FT // 2 - 1),
                                    perf_mode=DR)
                        nc.scalar.copy(rst[:, fo, sl], ps[:])
                        nc.vector.bn_stats(stats6[:, fo, ch * 6:(ch + 1) * 6],
                                           rst[:, fo, sl])

            def stats_phase(li, stats6):
                """bn_aggr -> local (mu, var) -> AllReduce(sum, sumsq) -> a, b."""
                muvar = work.tile([P, FT, 2], dt.float32, tag="acc", bufs=3,
                                  name=f"mv{li}")
                for ft in range(FT):
                    nc.vector.bn_aggr(muvar[:, ft, :], stats6[:, ft, :])
                statio = work.tile([P, 2 * FT], dt.float32, tag="acc", bufs=3,
                                   name=f"sio{li}")
                # sum = mu*Nc ; sumsq = (var + mu^2)*Nc
                tmp = work.tile([P, FT], dt.float32, tag="acc2", bufs=3,
                                name=f"tmp{li}")
                nc.vector.tensor_scalar(statio[:, :FT], muvar[:, :, 0], float(Nc),
                                        None, ALU.mult)
                nc.vector.tensor_tensor(tmp[:], muvar[:, :, 0], muvar[:, :, 0],
                                        ALU.mult)
                nc.vector.tensor_tensor(tmp[:], muvar[:, :, 1], tmp[:], ALU.add)
                nc.vector.tensor_scalar(statio[:, FT:], tmp[:], float(Nc),
                                        None, ALU.mult)
                nc.gpsimd.dma_start(stat_in[li - 1][:], statio[:])
                collective("AllReduce", ALU.add, [stat_in[li - 1]],
                           [stat_out[li - 1]])
                sums = work.tile([P, 2 * FT], dt.float32, tag="sums", bufs=1,
                                 name=f"sm{li}")
                nc.gpsimd.dma_start(sums[:], stat_out[li - 1][:])
                mu = work.tile([P, FT], dt.float32, tag="acc", bufs=3,
                               name=f"mu{li}")
                var = work.tile([P, FT], dt.float32, tag="acc", bufs=3,
                                name=f"vr{li}")
                nc.vector.tensor_scalar(mu[:], sums[:, :FT], 1.0 / N, None,
                                        ALU.mult)
                nc.vector.tensor_scalar(var[:], sums[:, FT:], 1.0 / N, None,
                                        ALU.mult)
                tm2 = work.tile([P, FT], dt.float32, tag="acc2", bufs=3,
                                name=f"tm{li}")
                nc.vector.tensor_tensor(tm2[:], mu[:], mu[:], ALU.mult)
                nc.vector.tensor_tensor(var[:], var[:], tm2[:], ALU.subtract)
                nc.vector.tensor_scalar(var[:], var[:], EPS, None, ALU.add)
                std = work.tile([P, FT], dt.float32, tag="acc2", bufs=3,
                                name=f"sd{li}")
                nc.scalar.activation(std[:], var[:], ACT.Sqrt)
                rstd = work.tile([P, FT], dt.float32, tag="acc2", bufs=3,
                                 name=f"rs{li}")
                nc.vector.reciprocal(rstd[:], std[:])
                a_sb = work.tile([P, FT], dt.float32, tag="ab", bufs=2,
                                 name=f"a{li}")
                b_sb = work.tile([P, FT], dt.float32, tag="ab", bufs=2,
                                 name=f"b{li}")
                nc.vector.tensor_tensor(a_sb[:], rstd[:], bn_t[f"bn{li}g"][:],
                                        ALU.mult)
                nc.vector.tensor_tensor(tm2[:], mu[:], a_sb[:], ALU.mult)
                nc.vector.tensor_tensor(b_sb[:], bn_t[f"bn{li}b"][:], tm2[:],
                                        ALU.subtract)
                return a_sb, b_sb

            def bn_apply_phase(li, rst, a_sb, b_sb, y_new, y3t):
                """y = prelu(a*rst + b); transpose to node-major (fp8);
                li<3 -> ynm HBM, li==3 -> y3t SBUF."""
                for ft in range(FT):
                    nc.scalar.activation(y_new[:, ft, :], rst[:, ft, :],
                                         ACT.Prelu,
                                         bias=b_sb[:, ft:ft + 1],
                                         scale=a_sb[:, ft:ft + 1],
                                         alpha=SLOPE)
                for nt in range(NT):
                    if li < 3:
                        yT = work.tile([P, HID], dt.float8e4, tag="yT", bufs=2,
                                       name=f"yT{li}_{nt}")
                    for fh in range(2):
                        # fp8 transpose writes with element step 2 (hw rule)
                        tp = psp.tile([P, 2048], dt.float8e4, tag="ps",
                                      name=f"ytp{li}_{nt}_{fh}")
                        for j in range(8):
                            ft = fh * 8 + j
                            o = tp[:, j * 256:(j + 1) * 256].rearrange(
                                "p (n two) -> p n two", two=2)[:, :, 0]
                            nc.tensor.transpose(o,
                                                y_new[:, ft, nt * P:(nt + 1) * P],
                                                ident_f8[:])
                        dst = (yT[:, fh * 1024:(fh + 1) * 1024] if li < 3
                               else y3t[:, nt, fh * 1024:(fh + 1) * 1024])
                        src = tp.rearrange("p (blk n two) -> p blk n two",
                                           blk=8, two=2)[:, :, :, 0]
                        evac(dst, src)
                    if li < 3:
                        nc.gpsimd.dma_start(ynm[nt * P:(nt + 1) * P, :], yT[:])
                        # pipelined allgather substitute: publish the local
                        # slice of this node tile immediately
                        if NOCC:
                            nc.gpsimd.dma_start(
                                yfull[li - 1][nt * P:(nt + 1) * P, :],
                                ynm[nt * P:(nt + 1) * P, :])

            # ---------------- the network ----------------
            ym = {}
            def ym_tile(name):
                return work.tile([P, FT, Nc], dt.float8e4, tag="ym", bufs=2,
                                 name=name)

            y3t = None
            done = False
            for li in (1, 2, 3):
                stats6 = work.tile([P, FT, NCH * 6], dt.float32, tag="st6",
                                   bufs=1, name=f"st6_{li}")
                rst = work.tile([P, FT, Nc], dt.bfloat16, tag="rst", bufs=1,
                                name=f"rst{li}")
                m_fm = None if li == 1 else ym_tile(f"m{li}")
                for g in range(NG // 2):
                    agg_group(li, m_fm, g)
                if STAGE == f"agg{li}":
                    done = True
                    break
                dense_phase(li, ym.get("y"), m_fm, rst, stats6, 0)
                dense_phase(li, ym.get("y"), m_fm, rst, stats6, 1)
                if STAGE == f"dense{li}":
                    done = True
                    break
                a_sb, b_sb = stats_phase(li, stats6)
                y_new = ym_tile(f"y{li}")
                if li == 3:
                    y3t = ym_tile("y3t")
                bn_apply_phase(li, rst, a_sb, b_sb, y_new, y3t)
                ym["y"] = y_new
                if STAGE == f"bn{li}":
                    done = True
                    break
                if li < 3 and not NOCC:
                    collective("AllGather", ALU.bypass, [ynm], [yfull[li - 1]])

            if STAGE == "full" and not done:
                # stage MLP head weights into dead SBUF (rst of layer 3 and a
                # spare ym slot) while bn3/pooling run, so the head is DMA-free
                for fo in range(FT):
                    nc.sync.dma_start(rst[:, fo, :], t_w["wf1"][:, fo])
                yw = ym_tile("yw").bitcast(dt.bfloat16)   # [P, FT, Nc//2] bf16
                for fo in range(MT):
                    nc.sync.dma_start(yw[:, 2 * fo:2 * fo + 2, :1024],
                                      t_w["wf2"][:, fo])

                def wf1_col(fo):
                    return rst[:, fo, :].rearrange("p (k m) -> p k m", k=FT)

                def wf2_col(fo):
                    return yw[:, 2 * fo:2 * fo + 2, :1024].rearrange(
                        "p a (k m) -> p (a k) m", k=FT // 2)

                # ---------------- pooling (feature-major) ----------------
                pps = [psp.tile([P, 512], dt.float32, tag="ps", name=f"pps{j}")
                       for j in range(2)]
                for ft in range(FT):
                    reg = pps[ft // 8][:, (ft % 8) * G:(ft % 8 + 1) * G]
                    for i in range(NT // 2):
                        nc.tensor.matmul(
                            reg,
                            lhsT=y3t[:, 2 * i:2 * i + 2, ft * P:(ft + 1) * P],
                            rhs=pmat_t[:, 2 * i:2 * i + 2, :],
                            start=(i == 0), stop=(i == NT // 2 - 1),
                            perf_mode=DR, skip_group_check=True)
                pool_sb = work.tile([P, FT, G], dt.float32, tag="pool", bufs=1)
                for j in range(2):
                    evac(pool_sb[:, j * 8:(j + 1) * 8, :], pps[j][:])
                nc.gpsimd.dma_start(pool_in[:], pool_sb[:])
                collective("AllReduce", ALU.add, [pool_in], [pool_out])
                hgsum = work.tile([P, FT, G], dt.float32, tag="pool2", bufs=1)
                nc.gpsimd.dma_start(hgsum[:], pool_out[:])
                hg_bf = work.tile([P, FT, G], dt.bfloat16, tag="hg", bufs=1)
                for ft in range(FT):
                    nc.vector.tensor_tensor(hg_bf[:, ft, :], hgsum[:, ft, :],
                                            invcnt_t[:], ALU.mult)

                # ---------------- MLP head (bf16) ----------------
                def fc_layer(wcol_fn, kt_count, fo_count, xin, bias_t, name):
                    xout = work.tile([P, fo_count, G], dt.bfloat16,
                                     tag=f"x{name}", bufs=1, name=f"x{name}")
                    for fo in range(fo_count):
                        wc = wcol_fn(fo)
                        ps = psp.tile([P, 512], dt.float32, tag="ps",
                                      name=f"hps{name}_{fo}")
                        for k in range(kt_count):
                            nc.tensor.matmul(ps[:, :G], lhsT=wc[:, k, :],
                                             rhs=xin[:, k, :],
                                             start=(k == 0),
                                             stop=(k == kt_count - 1))
                        nc.scalar.activation(xout[:, fo, :], ps[:, :G],
                                             ACT.Prelu,
                                             bias=bias_t[:, fo:fo + 1],
                                             scale=1.0, alpha=SLOPE)
                    return xout

                x1 = fc_layer(wf1_col, FT, FT, hg_bf, bn_t["f1b"], "f1")
                x2 = fc_layer(wf2_col, FT, MT, x1, bn_t["f2b"], "f2")

                ps18 = psp.tile([P, 512], dt.float32, tag="ps", name="ps18")
                for k in range(MT):
                    nc.tensor.matmul(ps18[:NCLS, :G], lhsT=wf3_t[:, k, :],
                                     rhs=x2[:, k, :], start=(k == 0),
                                     stop=(k == MT - 1))
                o18 = work.tile([NCLS, G], dt.float32, tag="o18", bufs=1)
                nc.vector.tensor_scalar(o18[:], ps18[:NCLS, :G], f3b_t[:, 0:1],
                                        None, ALU.add)
                tp = psp.tile([P, 256], dt.bfloat16, tag="ps", name="otp")
                tpf = tp.bitcast(dt.float32)
                nc.tensor.transpose(tpf[:G, :NCLS], o18[:], ident_f32[:NCLS, :NCLS])
                osb = work.tile([G, NCLS], dt.float32, tag="osb", bufs=1)
                nc.vector.tensor_copy(osb[:], tpf[:G, :NCLS])
                chn = work.tile([G, NCLS], dt.float32, tag="chn", bufs=1)
                nc.sync.dma_start(chn[:], t_chain[:])
                nc.vector.tensor_scalar(chn[:], chn[:], 0.0, None, ALU.mult)
                nc.vector.tensor_tensor(osb[:], osb[:], chn[:], ALU.add)
                nc.sync.dma_start(t_out[:], osb[:])
            else:
                # debug early-exit: emit *something* into out
                dbg = work.tile([G, NCLS], dt.float32, tag="dbg", bufs=1)
                nc.vector.memset(dbg[:], 0.0)
                nc.sync.dma_start(t_out[:], dbg[:])

    nc.compile()
    return nc


# --------------------------------------------------------------------------
# entry point
# --------------------------------------------------------------------------

LAST_EXEC_NS = None
LAST_TRACE = None


def _run_timed(nc, in_maps, iters=4, reps=None):
    """Mirror bass2jax.run_bass_via_pjrt but keep inputs device-resident so
    warm re-executions measure the on-device program span."""
    import time
    import jax
    import jax.numpy as jnp
    from jax.sharding import Mesh, PartitionSpec
    from jax.experimental.shard_map import shard_map
    import concourse.mybir as mybir
    from concourse.bass2jax import (
        install_neuronx_cc_hook, _bass_exec_p, partition_id_tensor)

    install_neuronx_cc_hook()
    n_cores = len(in_maps)
    partition_name = nc.partition_id_tensor.name if nc.partition_id_tensor else None
    in_names, out_names, out_avals, zero_outs = [], [], [], []
    for alloc in nc.m.functions[0].allocations:
        if not isinstance(alloc, mybir.MemoryLocationSet):
            continue
        name = alloc.memorylocations[0].name
        if alloc.kind == "ExternalInput":
            if name != partition_name:
                in_names.append(name)
        elif alloc.kind == "ExternalOutput":
            shape = tuple(alloc.tensor_shape)
            dtype = mybir.dt.np(alloc.dtype)
            out_names.append(name)
            out_avals.append(jax.core.ShapedArray(shape, dtype))
            zero_outs.append(np.zeros((n_cores * shape[0], *shape[1:]), dtype))
    n_params = len(in_names)
    all_in = list(in_names) + list(out_names)
    if partition_name is not None:
        all_in.append(partition_name)

    import os
    if reps is None:
        reps = int(os.environ.get("GCN_REPS", "1"))

    chain_idx = in_names.index("chain") if "chain" in in_names else None
    out_idx = out_names.index("out") if "out" in out_names else None

    def _body(*args):
        operands = list(args)
        if partition_name is not None:
            operands.append(partition_id_tensor())
        for _ in range(reps):
            outs = _bass_exec_p.bind(
                *operands, out_avals=tuple(out_avals), in_names=tuple(all_in),
                out_names=tuple(out_names), lowering_input_output_aliases=(),
                sim_require_finite=True, sim_require_nnan=True, nc=nc)
            if chain_idx is not None and out_idx is not None:
                operands[chain_idx] = outs[out_idx]
        return tuple(outs)

    devices = jax.devices()[:n_cores]
    mesh = Mesh(np.asarray(devices), ("core",))
    nin = n_params + len(out_names)
    sharded = jax.jit(
        shard_map(_body, mesh=mesh, in_specs=(PartitionSpec("core"),) * nin,
                  out_specs=(PartitionSpec("core"),) * len(out_names),
                  check_rep=False),
        donate_argnums=tuple(range(n_params, nin)), keep_unused=True)

    shd = jax.sharding.NamedSharding(mesh, PartitionSpec("core"))
    dev_in = [
        jax.device_put(
            np.concatenate([np.asarray(in_maps[c][nm]) for c in range(n_cores)],
                           axis=0), shd)
        for nm in in_names
    ]
    times = []
    outs = None
    for _ in range(iters):
        zo = [jax.device_put(z.copy(), shd) for z in zero_outs]
        for z in zo:
            z.block_until_ready()
        t0 = time.perf_counter()
        outs = sharded(*dev_in, *zo)
        for o in outs:
            o.block_until_ready()
        times.append(time.perf_counter() - t0)
    best_ns = int(min(times) * 1e9 / reps)
    results = [
        {nm: np.asarray(outs[i]).reshape(n_cores, *out_avals[i].shape)[c]
         for i, nm in enumerate(out_names)}
        for c in range(n_cores)
    ]
    print(f"timed runs (s, reps={reps}): {[f'{t:.4f}' for t in times]}")
    return results, best_ns


def kernel(**inputs) -> np.ndarray:
    global LAST_EXEC_NS, LAST_TRACE
    from concourse.bass_utils import run_bass_kernel_spmd

    import os

    in_maps, meta = _preprocess(inputs)
    nc = _build(meta)
    in_maps = [{k: np.ascontiguousarray(v) for k, v in m.items()}
               for m in in_maps]
    if os.environ.get("GCN_TIME"):
        results, best_ns = _run_timed(nc, in_maps)
        LAST_EXEC_NS = best_ns
        return np.asarray(results[0]["out"], np.float32)
    res = run_bass_kernel_spmd(nc, in_maps, core_ids=list(range(C)))
    LAST_EXEC_NS = res.exec_time_ns
    LAST_TRACE = res.instructions_and_trace
    return np.asarray(res.results[0]["out"], np.float32)
